# revision 55
# baseline (speedup 1.0000x reference)
"""Trainium2 Bass kernel for nn_DecoderV1 (dilated-conv decoder, 24-step recurrence).

Strategy: pure data parallel over batch (2048 -> 8 cores x 256). Inside a core,
activations live channel-major ([ch, batch] on [partitions, free]) in bf16; the
24x6 (step x layer) recurrence is emitted wavefront-ordered (w = t + l) as
straight-line Tile code so independent blocks pipeline across engines.

Engine assignment (hardware-legal: gpsimd/Pool CANNOT read PSUM, and SBUF+SBUF
DVE operands must share a base partition):
  Act : tanh per block-group, g12/g34/g5 skip-relus, h-relu, final y evac
  DVE : gating ts+tt (fast/g12/g34), res updates (stt from PSUM), fast relu
  Pool: g5 gating only (the one SBUF-only op pair)
  PE  : W2/W3/W4 matmuls, W5 x6 accumulate, W6-variant accumulate into yp

Key optimizations over the naive wavefront emit:
 - W6 y-outputs accumulate into one persistent [12,512] PSUM bank via 12
   host-built weight variants (only col k nonzero), evac'd by ONE Act op at
   the end instead of 12 [1,512] evacs (-6.7us Act).
 - All 64-partition constants ride in ONE packed dram tensor; 7 large DMAs
   (HWDGE costs 625ns fixed per descriptor) ordered critical-first so the
   fast chain unblocks ~2.7us after start instead of ~7.5us.
 - Priorities: fast block at w*100-99 so its ops win scheduler tie-breaks
   against the previous wavefront's group res/relu ops (the d=1 chain is the
   only 1-wavefront recurrence cycle); slack ops (skip-relus, h/W5/W6)
   deferred +2 wavefronts to fill pipeline bubbles; group order g12, g5, g34.
x0 = tanh(W1@[init,feat]+b1) is precomputed on the host (no recurrence).
Only the encoder tail (last d columns per dilation d) is ever read, so the
host feeds 2MB instead of 528MB.
"""
import numpy as np
import ml_dtypes

DIL = (1, 2, 4, 8, 16, 32)
NSLOT = 63
T = 24
B = 2048
NC = 8
BL = B // NC          # 256 batch per core
F = 64                # filters
NW = T + len(DIL) - 1  # 29 wavefronts

# packed-constants layout (cols of the [64, PK_TOTAL] bf16 dram tensor)
PK_W2 = 0
PK_W3 = 128
PK_W4 = 256
PK_X0A = 640            # x0 steps 0-1
PK_D1 = 1152            # end of critical first DMA
PK_W5 = 1152
PK_X0B = 3456           # x0 steps 2-7
PK_D2 = 4992
PK_X0C = 7040           # x0 steps 8-23
PK_D3 = 11136
PK_D4 = 15232
PK_TOTAL = 23424
PK_CIRC = {0: 384, 1: 1920, 2: 2432, 3: 4992, 4: 11136, 5: 15232}

_CACHE = {}


def _build():
    import concourse.bass as bass
    import concourse.tile as tile
    import concourse.mybir as mybir

    F32, BF16 = mybir.dt.float32, mybir.dt.bfloat16
    AF = mybir.ActivationFunctionType
    OP = mybir.AluOpType

    nc = bass.Bass("TRN2", target_bir_lowering=False, debug=False)

    # all 64-partition constants live in ONE packed dram tensor so HWDGE
    # issues few large DMAs (625ns fixed overhead each) instead of 20 small
    # ones; layout must match _prep_inputs PK_* offsets
    pk_d = nc.dram_tensor("pk", [F, PK_TOTAL], BF16, kind="ExternalInput")
    bpk_d = nc.dram_tensor("bpk", [128, 5], F32, kind="ExternalInput")
    w6_d = nc.dram_tensor("w6", [128, 12 * 12], BF16, kind="ExternalInput")
    y_d = nc.dram_tensor("y", [12, 2 * BL], F32, kind="ExternalOutput")

    with tile.TileContext(nc) as tc:
        with tc.tile_pool(name="const", bufs=1) as cpool, \
             tc.tile_pool(name="work", bufs=1) as wpool, \
             tc.tile_pool(name="psum", bufs=1, space="PSUM") as ppool:

            pk_t = cpool.tile([F, PK_TOTAL], BF16, name="pk")
            bpk_t = cpool.tile([128, 5], F32, name="bpk")
            w6 = cpool.tile([128, 12 * 12], BF16)
            ring = cpool.tile([F, 6 * 6 * BL], BF16)   # slab = (w%6)*1536
            y_all = cpool.tile([12, 2 * BL], F32)
            yp = ppool.tile([12, 2 * BL], F32, name="yp")

            w2 = pk_t[:, PK_W2:PK_W2 + 128]
            w3 = pk_t[:, PK_W3:PK_W3 + 128]
            w4 = pk_t[:, PK_W4:PK_W4 + 128]
            w5 = pk_t[:, PK_W5:PK_W5 + 768]
            circs = [pk_t[:, PK_CIRC[l]:PK_CIRC[l] + DIL[l] * BL]
                     for l in range(6)]
            b2 = bpk_t[:, 0:1]
            b4 = bpk_t[:, 1:2]
            b5 = bpk_t[:, 2:3]
            b4r = bpk_t[0:64, 3:4]
            b6 = bpk_t[0:12, 4:5]

            def x0v(t):
                # x0 chunk views: t in [0,2) / [2,8) / [8,24)
                if t < 2:
                    base = PK_X0A + t * BL
                elif t < 8:
                    base = PK_X0B + (t - 2) * BL
                else:
                    base = PK_X0C + (t - 8) * BL
                return pk_t[:, base:base + BL]

            # DMA order: critical-first packs. D1 carries w2/w3/w4/circ0/x0[0:2]
            # so the fast chain unblocks after 2 descriptors (~1.3us).
            nc.sync.dma_start(pk_t[:, 0:PK_D1], pk_d.ap()[:, 0:PK_D1])
            nc.sync.dma_start(bpk_t[:], bpk_d.ap())
            nc.sync.dma_start(pk_t[:, PK_D1:PK_D2], pk_d.ap()[:, PK_D1:PK_D2])
            nc.sync.dma_start(w6[:], w6_d.ap())
            nc.sync.dma_start(pk_t[:, PK_D2:PK_D3], pk_d.ap()[:, PK_D2:PK_D3])
            nc.sync.dma_start(pk_t[:, PK_D3:PK_D4], pk_d.ap()[:, PK_D3:PK_D4])
            nc.sync.dma_start(pk_t[:, PK_D4:PK_TOTAL],
                              pk_d.ap()[:, PK_D4:PK_TOTAL])

            from contextlib import contextmanager

            @contextmanager
            def prio(p):
                save = tc.cur_priority
                tc.cur_priority = p
                yield
                tc.cur_priority = save

            # x-history: xh[w][l*BL:(l+1)*BL] holds x_l for step t=w-l-1
            xh_tiles = {}

            def get_xh(w):
                if w not in xh_tiles:
                    xh_tiles[w] = wpool.tile([F, 5 * BL], BF16, tag="xh", bufs=17,
                                             name=f"xh{w}")
                return xh_tiles[w]

            def state_src(l, t):
                if t < DIL[l]:
                    return circs[l][:, t * BL:(t + 1) * BL]
                wsrc = (t - DIL[l]) + l + 1
                return xh_tiles[wsrc][:, l * BL:(l + 1) * BL]

            def x_src(l, t, w):
                # x_{l-1}^t
                if l == 0:
                    return x0v(t)
                return get_xh(w)[:, (l - 1) * BL:l * BL]

            for w in range(NW):
                lmin, lmax = max(0, w - (T - 1)), min(5, w)
                has0 = lmin == 0

                th = wpool.tile([128, 6 * BL], BF16, tag="th", bufs=8, name=f"th{w}")
                ssb = wpool.tile([F, 6 * BL], BF16, tag="ssb", bufs=8, name=f"ss{w}")
                gated = wpool.tile([F, 6 * BL], BF16, tag="gated", bufs=8,
                                   name=f"gt{w}")
                rbase = (w % 6) * (6 * BL)

                # fast path: block 0 (the d=1 recurrence chain). The PSUM tile
                # doubles as dc then outp: W4 overwrites after tanh read.
                if has0:
                    t = w
                    with prio(-1000000 + w * 100 - 99):
                        pf = ppool.tile([128, BL], F32, tag="pf", bufs=1,
                                        name=f"pf{w}")
                        nc.tensor.matmul(pf[:], w2, state_src(0, t),
                                         start=True, stop=False)
                        nc.tensor.matmul(pf[:], w3, x_src(0, t, w),
                                         start=False, stop=True)
                        nc.scalar.activation(th[:, 0:BL], pf[:], AF.Tanh,
                                             bias=b2)
                        # gated = (th_g + 1) * th_f  (the 0.5 lives in W4)
                        nc.vector.tensor_scalar(out=ssb[:, 0:BL],
                                                in0=th[64:128, 0:BL],
                                                scalar1=1.0, scalar2=1.0,
                                                op0=OP.mult, op1=OP.add)
                        nc.vector.tensor_tensor(out=gated[:, 0:BL],
                                                in0=th[0:64, 0:BL],
                                                in1=ssb[:, 0:BL], op=OP.mult)
                        nc.tensor.matmul(pf[:], w4, gated[:, 0:BL],
                                         start=True, stop=True)
                        if w + 1 < NW and w <= T - 1:
                            nxh = get_xh(w + 1)
                            nc.vector.scalar_tensor_tensor(
                                out=nxh[:, 0:BL], in0=pf[64:128, :],
                                scalar=b4r, in1=x0v(w),
                                op0=OP.add, op1=OP.add)
                    with prio(-1000000 + (w + 2) * 100 + 70):
                        nc.vector.tensor_scalar(
                            out=ring[:, rbase:rbase + BL], in0=pf[0:64, :],
                            scalar1=b4[0:64, :], scalar2=0.0,
                            op0=OP.add, op1=OP.max)

                # rest blocks in layer-groups: each group is an independent
                # per-wavefront chain (W3 -> tanh -> ssb -> tt -> W4 -> res)
                # with disjoint column halves of a shared double-buffered PSUM
                # tile (g12 cols 0:512, g34 cols 512:1024) so the slack relu
                # can cover both groups in one Act op; g5 has its own tile
                r0 = max(1, lmin)
                pgA = None
                for grp in ((1, 2), (3, 4), (5,)):
                    gl = [l for l in grp if r0 <= l <= lmax]
                    if not gl:
                        continue
                    g0, g1 = gl[0], gl[-1]
                    gw = (g1 + 1 - g0) * BL
                    gc0, gc1 = g0 * BL, (g1 + 1) * BL
                    with prio(-1000000 + w * 100 + {(1, 2): 2, (3, 4): 10, (5,): 4}[grp]):
                        pg = ppool.tile([128, len(grp) * BL], F32,
                                        tag=f"pg{grp[0]}",
                                        bufs=1 if grp == (5,) else 2,
                                        name=f"pg{grp[0]}_{w}")
                        po = 0
                        if grp != (5,):
                            pgA = (pgA or [])
                            pgA.append((pg, gl))
                        xs = get_xh(w)
                        for l in gl:
                            t = w - l
                            lc = po + (l - g0) * BL
                            nc.tensor.matmul(pg[:, lc:lc + BL],
                                             w2, state_src(l, t),
                                             start=True, stop=False)
                            nc.tensor.matmul(pg[:, lc:lc + BL], w3,
                                             xs[:, (l - 1) * BL:l * BL],
                                             start=False, stop=True)
                        nc.scalar.activation(th[:, gc0:gc1], pg[:, po:po + gw],
                                             AF.Tanh, bias=b2)
                        # gated = (th_g + 1) * th_f; g5 on Pool (SBUF-only
                        # ops are the only legal gpsimd work), rest on DVE
                        eng = nc.gpsimd if grp == (5,) else nc.vector
                        eng.tensor_scalar(out=ssb[:, gc0:gc1],
                                          in0=th[64:128, gc0:gc1],
                                          scalar1=1.0, scalar2=1.0,
                                          op0=OP.mult, op1=OP.add)
                        eng.tensor_tensor(out=gated[:, gc0:gc1],
                                          in0=th[0:64, gc0:gc1],
                                          in1=ssb[:, gc0:gc1],
                                          op=OP.mult)
                        nc.tensor.matmul(pg[:, po:po + gw], w4,
                                         gated[:, gc0:gc1],
                                         start=True, stop=True)
                        lf1 = min(4, g1)
                        if lf1 >= g0 and w + 1 < NW:
                            nxh = get_xh(w + 1)
                            nc.vector.scalar_tensor_tensor(
                                out=nxh[:, g0 * BL:(lf1 + 1) * BL],
                                in0=pg[64:128, po:po + (lf1 + 1 - g0) * BL],
                                scalar=b4r,
                                in1=get_xh(w)[:, (g0 - 1) * BL:lf1 * BL],
                                op0=OP.add, op1=OP.add)
                    if grp == (5,):
                        with prio(-1000000 + (w + 2) * 100 + 72):
                            nc.scalar.activation(
                                ring[:, rbase + gc0:rbase + gc1],
                                pg[0:64, 0:gw], AF.Relu, bias=b4[0:64, :])
                # skips relu per g12/g34 group -> ring, on Pool (Act is
                # tanh-bound); low priority so it sorts after wavefront w+1
                # chain ops
                if pgA is not None:
                    for pg_, gl_ in pgA:
                        b0, b1_ = gl_[0], gl_[-1]
                        with prio(-1000000 + (w + 2) * 100 + 74 + b0):
                            nc.scalar.activation(
                                ring[:, rbase + b0 * BL:rbase + (b1_ + 1) * BL],
                                pg_[0:64, 0:(b1_ + 1 - b0) * BL], AF.Relu,
                                bias=b4[0:64, :])

                # step completion, batched per 2 steps (s odd): hp/yp hold both
                s = w - 5
                if s >= 0:
                  with prio(-1000000 + (w + 2) * 100 + 80):
                    hp = ppool.tile([128, 2 * BL], F32, tag="hp", bufs=1,
                                    name=f"hp{s // 2}") if s % 2 == 0 else hp
                    hoff = (s % 2) * BL
                    for l in range(6):
                        base = ((s + l) % 6) * (6 * BL)
                        nc.tensor.matmul(hp[:, hoff:hoff + BL],
                                         w5[:, l * 128:(l + 1) * 128],
                                         ring[:, base + l * BL:base + (l + 1) * BL],
                                         start=(l == 0), stop=(l == 5))
                    if s % 2 == 1 or s == T - 1:
                        n = BL if (s % 2 == 0) else 2 * BL
                        hsb = wpool.tile([128, 2 * BL], BF16, tag="hsb", bufs=3,
                                         name=f"hs{s // 2}")
                        nc.scalar.activation(hsb[:, 0:n], hp[:, 0:n], AF.Relu,
                                             bias=b5)
                        # W6 variant k (only col k nonzero) accumulates y into
                        # partition k of the persistent yp psum bank; rows != k
                        # get +0. One Act evac for all 24 steps after the loop.
                        k = s // 2
                        nc.tensor.matmul(yp[:, 0:n],
                                         w6[:, 12 * k:12 * (k + 1)],
                                         hsb[:, 0:n],
                                         start=(k == 0), stop=(k == 11))

            nc.scalar.activation(y_all[:], yp[:], AF.Identity, bias=b6)
            nc.sync.dma_start(y_d.ap(), y_all[:])

    _split_multi_waits(nc)
    return nc


def _split_multi_waits(nc, max_waits: int = 1) -> int:
    """This walrus build encodes at most one sync wait per instruction; hoist
    extras onto same-engine EventSemaphore wait-nops (as raw bass emits)."""
    import concourse.mybir as mybir
    n = 0
    for f in nc.m.functions:
        for bb in f.blocks:
            insts = bb.instructions
            if not any(i.sync_info and i.sync_info.on_wait
                       and len(i.sync_info.on_wait) > max_waits for i in insts):
                continue
            new = []
            for inst in insts:
                si = inst.sync_info
                if si is not None and si.on_wait and len(si.on_wait) > max_waits:
                    waits = list(si.on_wait)
                    for j, wt in enumerate(waits[:-max_waits]):
                        new.append(mybir.InstEventSemaphore(
                            name=f"{inst.name}_xw{j}", engine=inst.engine,
                            sync_info=mybir.SyncInfo(on_wait=[wt], on_update=[])))
                        n += 1
                    inst.sync_info = mybir.SyncInfo(
                        on_wait=waits[-max_waits:], on_update=list(si.on_update))
                new.append(inst)
            bb.instructions = new
    return n


def _prep_inputs(inputs):
    bf = ml_dtypes.bfloat16
    enc = np.asarray(inputs["encoder_outputs"], np.float32)   # [6,2048,168,64]
    df = np.asarray(inputs["decoder_features"], np.float32)   # [2048,24,15]
    di = np.asarray(inputs["decoder_init_input"], np.float32)  # [2048,1]

    W = {k: np.asarray(inputs[k], np.float32) for k in
         ["W1", "W2", "W3", "W4", "W5", "W6", "b1", "b2", "b4", "b5", "b6"]}
    w2 = W["W2"].copy(); w2[:, 64:128] *= 0.5   # sigmoid(g)=0.5*tanh(g/2)+0.5
    w3 = W["W3"].copy(); w3[:, 64:128] *= 0.5
    w4 = 0.5 * W["W4"]                          # gated computed as 2x true value
    w5 = np.concatenate([W["W5"][l * 64:(l + 1) * 64, :] for l in range(6)],
                        axis=1).astype(bf)                            # [64, 768]
    b2 = W["b2"].reshape(128, 1).astype(np.float32).copy(); b2[64:128] *= 0.5
    b4 = W["b4"].reshape(128, 1).astype(np.float32)
    b5 = W["b5"].reshape(128, 1).astype(np.float32)
    b6 = np.full((12, 1), float(W["b6"].reshape(-1)[0]), np.float32)
    w6v = np.zeros((128, 12 * 12), np.float32)
    for k in range(12):
        w6v[:, 12 * k + k] = W["W6"][:, 0]
    w6v = w6v.astype(bf)

    # host-precomputed x0 = tanh([init, feat_t] @ W1 + b1)  -> [B, T, 64]
    this_in = np.concatenate(
        [np.repeat(di[:, None, :], T, axis=1), df], axis=2)   # [B, T, 16]
    x0 = np.tanh(this_in @ W["W1"] + W["b1"])                 # [B, T, 64]

    bpk = np.zeros((128, 5), np.float32)
    bpk[:, 0:1] = b2
    bpk[:, 1:2] = b4
    bpk[:, 2:3] = b5
    bpk[0:64, 3:4] = b4[64:128]
    bpk[0:12, 4:5] = b6

    in_maps = []
    for c in range(NC):
        bs = slice(c * BL, (c + 1) * BL)
        pk = np.zeros((F, PK_TOTAL), bf)
        pk[:, PK_W2:PK_W2 + 128] = w2.astype(bf)
        pk[:, PK_W3:PK_W3 + 128] = w3.astype(bf)
        pk[:, PK_W4:PK_W4 + 128] = w4.astype(bf)
        pk[:, PK_W5:PK_W5 + 768] = w5
        x0c = np.ascontiguousarray(
            x0[bs].transpose(2, 1, 0)).reshape(F, T * BL).astype(bf)
        pk[:, PK_X0A:PK_X0A + 2 * BL] = x0c[:, 0:2 * BL]
        pk[:, PK_X0B:PK_X0B + 6 * BL] = x0c[:, 2 * BL:8 * BL]
        pk[:, PK_X0C:PK_X0C + 16 * BL] = x0c[:, 8 * BL:24 * BL]
        for l, d in enumerate(DIL):
            blk = np.ascontiguousarray(
                np.transpose(enc[l, bs, 168 - d:168, :], (2, 1, 0)))  # [F, d, BL]
            pk[:, PK_CIRC[l]:PK_CIRC[l] + d * BL] = \
                blk.reshape(F, d * BL).astype(bf)
        in_maps.append({"pk": pk, "bpk": bpk, "w6": w6v})
    return in_maps


def kernel(**inputs) -> np.ndarray:
    from concourse.bass_utils import run_bass_kernel_spmd
    if "nc" not in _CACHE:
        _CACHE["nc"] = _build()
    nc = _CACHE["nc"]
    in_maps = _prep_inputs(inputs)
    res = run_bass_kernel_spmd(nc, in_maps, core_ids=list(range(NC)))
    out = np.empty((B, T, 1), np.float32)
    for c in range(NC):
        y = res.results[c]["y"].reshape(12, 2, BL)  # [t//2, t%2, b]
        out[c * BL:(c + 1) * BL, :, 0] = y.reshape(T, BL).T
    return out

